# revision 22
# baseline (speedup 1.0000x reference)
"""DropoutDPP eval-path kernel for 8 Trainium2 NeuronCores.

The reference accumulates Bernoulli keep-masks (fixed RNG key 42, independent
of x) over the hidden dim until >=40% of neurons are nonzero, then computes
x * sum_mask / i.  The mask state is a deterministic constant, so it is
computed host-side (same jax threefry bits as the reference) and folded into a
single [hidden] scale vector.  The heavy, memory-bound part — scaling the
[4, 4096, 4096] tensor along its last dim — runs on 8 cores, data-parallel
over rows.

The on-device kernel is hand-scheduled raw Bass (this toolchain's TT struct
encodes a single sync wait, which rules out Tile's multi-wait scheduling):
SP issues the scale load then x-tile loads, DVE multiplies in place, ACT
issues stores; eight 1MB SBUF slots rotate, one semaphore per slot so wait
thresholds are exact.
"""

import numpy as np

_B, _S, _H = 4, 4096, 4096
_N_CORES = 8
_ROWS = _B * _S                       # 16384
_ROWS_PER_CORE = _ROWS // _N_CORES    # 2048
_P = 0.9
_MAX_N = 100
_MAX_FRAC = 0.4

_COLS_PER_TILE = 2048                 # [128, 2048] f32 = 1MB tiles
_TILES_PER_ROWBLK = _H // _COLS_PER_TILE
_N_ROWBLKS = _ROWS_PER_CORE // 128    # 16
_N_TILES = _N_ROWBLKS * _TILES_PER_ROWBLK  # 32
_N_SLOTS = 8

_cache: dict = {}


def _compute_scale() -> np.ndarray:
    """Replicate reference._accumulate_masks exactly (threefry is
    backend/platform deterministic), returning sum_mask / i as float32."""
    if "scale" in _cache:
        return _cache["scale"]
    import jax
    import jax.numpy as jnp

    cpu = jax.devices("cpu")[0]
    with jax.default_device(cpu):
        key = jax.random.key(42)
        key, k0 = jax.random.split(key)
        sum_mask = (jax.random.uniform(k0, (_H,)) >= _P).astype(jnp.float32)
        i = 1
        while i < _MAX_N and float(
            jnp.mean((sum_mask != 0).astype(jnp.float32))
        ) < _MAX_FRAC:
            key, k = jax.random.split(key)
            sum_mask = sum_mask + (jax.random.uniform(k, (_H,)) >= _P).astype(
                jnp.float32
            )
            i += 1
    scale = np.asarray(sum_mask, dtype=np.float32) / np.float32(i)
    _cache["scale"] = scale
    return scale


def _build_nc():
    if "nc" in _cache:
        return _cache["nc"]
    import concourse.bass as bass
    import concourse.mybir as mybir
    from contextlib import ExitStack

    nc = bass.Bass(trn_type="TRN2")
    x = nc.dram_tensor(
        "x", [_ROWS_PER_CORE, _H], mybir.dt.float32, kind="ExternalInput"
    )
    scale = nc.dram_tensor(
        "scale", [128, _H], mybir.dt.float32, kind="ExternalInput"
    )
    y = nc.dram_tensor(
        "y", [_ROWS_PER_CORE, _H], mybir.dt.float32, kind="ExternalOutput"
    )

    # row-block-major view; tiles slice columns out of a row block
    xv = x[:, :].rearrange("(n p) h -> n p h", p=128)
    yv = y[:, :].rearrange("(n p) h -> n p h", p=128)

    # (rowblk, col0, ncols): 2048-wide tiles, except the last row block is
    # split 4x1024 to shorten the end-of-pipeline load->mul->store chain
    tiles = []
    for n in range(_N_ROWBLKS):
        if n == _N_ROWBLKS - 1:
            tiles += [(n, c * 1024, 1024) for c in range(4)]
        else:
            tiles += [(n, c * _COLS_PER_TILE, _COLS_PER_TILE) for c in range(2)]

    with ExitStack() as ctx:
        scale_sb = ctx.enter_context(
            nc.sbuf_tensor("scale_sb", [128, _H], mybir.dt.float32)
        )
        slots = [
            ctx.enter_context(
                nc.sbuf_tensor(
                    f"slot{s}", [128, _COLS_PER_TILE], mybir.dt.float32
                )
            )
            for s in range(_N_SLOTS)
        ]
        # One semaphore per slot: each slot has at most one outstanding DMA
        # at a time (load +16, store +16 → +32 per slot cycle), making the
        # wait thresholds exact.  A single shared DMA sem would race: the 16
        # SDMA engines increment independently per transfer, so "sem >=
        # 16*(i+1)" does not imply transfers 0..i all completed.
        slot_sems = [
            ctx.enter_context(nc.semaphore(f"slot_sem{s}"))
            for s in range(_N_SLOTS)
        ]
        mul_sem = ctx.enter_context(nc.semaphore("mul_sem"))
        sc_sem = ctx.enter_context(nc.semaphore("sc_sem"))
        block = ctx.enter_context(nc.Block())

        n_tiles = len(tiles)
        slot_of = [i % _N_SLOTS for i in range(n_tiles)]
        use_of = [i // _N_SLOTS for i in range(n_tiles)]
        slot_uses = [
            sum(1 for i in range(n_tiles) if slot_of[i] == s)
            for s in range(_N_SLOTS)
        ]

        @block.sync
        def _(sync):
            # scale first on the same HWDGE queue, split into column halves:
            # FIFO per SDMA engine lands half 0 before L0, so mul0 isn't
            # gated on the full 2MB broadcast row
            half = _H // 2
            sync.dma_start(
                out=scale_sb[:, :half], in_=scale[:, :half]
            ).then_inc(sc_sem, 16)
            for i, (n, c0, w) in enumerate(tiles):
                s, use = slot_of[i], use_of[i]
                if use > 0:
                    sync.wait_ge(slot_sems[s], 32 * use)  # prev store landed
                sync.dma_start(
                    out=slots[s][:, :w], in_=xv[n, :, c0 : c0 + w]
                ).then_inc(slot_sems[s], 16)
                if i == 0:
                    # second scale half rides between L0 and L1 so L0 (and
                    # with it mul0/store0) isn't queued behind the full row
                    sync.dma_start(
                        out=scale_sb[:, half:], in_=scale[:, half:]
                    ).then_inc(sc_sem, 16)

        @block.vector
        def _(vector):
            cur_sc = 0
            for i, (n, c0, w) in enumerate(tiles):
                s, use = slot_of[i], use_of[i]
                need_sc = 16 if c0 + w <= _H // 2 else 32
                if need_sc > cur_sc:
                    vector.wait_ge(sc_sem, need_sc)
                    cur_sc = need_sc
                vector.wait_ge(slot_sems[s], 32 * use + 16)  # load landed
                t = slots[s]
                vector.tensor_mul(
                    out=t[:, :w], in0=t[:, :w], in1=scale_sb[:, c0 : c0 + w]
                ).then_inc(mul_sem, 1)

        @block.scalar
        def _(scalar):
            for i, (n, c0, w) in enumerate(tiles):
                s = slot_of[i]
                scalar.wait_ge(mul_sem, i + 1)
                scalar.dma_start(
                    out=yv[n, :, c0 : c0 + w], in_=slots[s][:, :w]
                ).then_inc(slot_sems[s], 16)
            # all output bytes landed before the program ends
            for s in range(_N_SLOTS):
                scalar.wait_ge(slot_sems[s], 32 * slot_uses[s])

    _cache["nc"] = nc
    return nc


def _run(x: np.ndarray, trace: bool = False, trace_cores=None):
    """Returns (full_output, BassKernelResults)."""
    from concourse.bass_utils import run_bass_kernel_spmd

    nc = _build_nc()
    scale_bc = np.ascontiguousarray(
        np.broadcast_to(_compute_scale()[None, :], (128, _H))
    )
    xf = np.ascontiguousarray(x, dtype=np.float32).reshape(_ROWS, _H)
    in_maps = [
        {"x": xf[c * _ROWS_PER_CORE : (c + 1) * _ROWS_PER_CORE], "scale": scale_bc}
        for c in range(_N_CORES)
    ]
    res = run_bass_kernel_spmd(
        nc,
        in_maps,
        core_ids=list(range(_N_CORES)),
        trace=trace,
        trace_cores=trace_cores,
    )
    out = np.concatenate([r["y"] for r in res.results], axis=0)
    return out.reshape(_B, _S, _H), res


def kernel(**inputs) -> np.ndarray:
    out, _ = _run(np.asarray(inputs["x"]))
    return out


# revision 26
# speedup vs baseline: 1.0102x; 1.0102x over previous
"""DropoutDPP eval-path kernel for 8 Trainium2 NeuronCores.

The reference accumulates Bernoulli keep-masks (fixed RNG key 42, independent
of x) over the hidden dim until >=40% of neurons are nonzero, then computes
x * sum_mask / i.  The mask state is a deterministic constant, so it is
computed host-side (same jax threefry bits as the reference) and folded into a
single [hidden] scale vector.  The heavy, memory-bound part — scaling the
[4, 4096, 4096] tensor along its last dim — runs on 8 cores, data-parallel
over rows.

The on-device kernel is hand-scheduled raw Bass (this toolchain's TT struct
encodes a single sync wait, which rules out Tile's multi-wait scheduling):
SP issues the scale load then x-tile loads, DVE multiplies in place, ACT
issues stores; eight 1MB SBUF slots rotate, one semaphore per slot so wait
thresholds are exact.
"""

import numpy as np

_B, _S, _H = 4, 4096, 4096
_N_CORES = 8
_ROWS = _B * _S                       # 16384
_ROWS_PER_CORE = _ROWS // _N_CORES    # 2048
_P = 0.9
_MAX_N = 100
_MAX_FRAC = 0.4

_COLS_PER_TILE = 2048                 # [128, 2048] f32 = 1MB tiles
_TILES_PER_ROWBLK = _H // _COLS_PER_TILE
_N_ROWBLKS = _ROWS_PER_CORE // 128    # 16
_N_TILES = _N_ROWBLKS * _TILES_PER_ROWBLK  # 32
_N_SLOTS = 8

_cache: dict = {}


def _compute_scale() -> np.ndarray:
    """Replicate reference._accumulate_masks exactly (threefry is
    backend/platform deterministic), returning sum_mask / i as float32."""
    if "scale" in _cache:
        return _cache["scale"]
    import jax
    import jax.numpy as jnp

    cpu = jax.devices("cpu")[0]
    with jax.default_device(cpu):
        key = jax.random.key(42)
        key, k0 = jax.random.split(key)
        sum_mask = (jax.random.uniform(k0, (_H,)) >= _P).astype(jnp.float32)
        i = 1
        while i < _MAX_N and float(
            jnp.mean((sum_mask != 0).astype(jnp.float32))
        ) < _MAX_FRAC:
            key, k = jax.random.split(key)
            sum_mask = sum_mask + (jax.random.uniform(k, (_H,)) >= _P).astype(
                jnp.float32
            )
            i += 1
    scale = np.asarray(sum_mask, dtype=np.float32) / np.float32(i)
    _cache["scale"] = scale
    return scale


def _build_nc():
    if "nc" in _cache:
        return _cache["nc"]
    import concourse.bass as bass
    import concourse.mybir as mybir
    from contextlib import ExitStack

    nc = bass.Bass(trn_type="TRN2")
    x = nc.dram_tensor(
        "x", [_ROWS_PER_CORE, _H], mybir.dt.float32, kind="ExternalInput"
    )
    scale = nc.dram_tensor(
        "scale", [128, _H], mybir.dt.float32, kind="ExternalInput"
    )
    y = nc.dram_tensor(
        "y", [_ROWS_PER_CORE, _H], mybir.dt.float32, kind="ExternalOutput"
    )

    # row-block-major view; tiles slice columns out of a row block
    xv = x[:, :].rearrange("(n p) h -> n p h", p=128)
    yv = y[:, :].rearrange("(n p) h -> n p h", p=128)

    # (rowblk, col0, ncols): uniform [128, 2048] 1MB tiles
    tiles = [
        (n, c * _COLS_PER_TILE, _COLS_PER_TILE)
        for n in range(_N_ROWBLKS)
        for c in range(_TILES_PER_ROWBLK)
    ]

    with ExitStack() as ctx:
        scale_sb = ctx.enter_context(
            nc.sbuf_tensor("scale_sb", [128, _H], mybir.dt.float32)
        )
        slots = [
            ctx.enter_context(
                nc.sbuf_tensor(
                    f"slot{s}", [128, _COLS_PER_TILE], mybir.dt.float32
                )
            )
            for s in range(_N_SLOTS)
        ]
        # One semaphore per slot: each slot has at most one outstanding DMA
        # at a time (load +16, store +16 → +32 per slot cycle), making the
        # wait thresholds exact.  A single shared DMA sem would race: the 16
        # SDMA engines increment independently per transfer, so "sem >=
        # 16*(i+1)" does not imply transfers 0..i all completed.
        slot_sems = [
            ctx.enter_context(nc.semaphore(f"slot_sem{s}"))
            for s in range(_N_SLOTS)
        ]
        mul_sem = ctx.enter_context(nc.semaphore("mul_sem"))
        sc_sem = ctx.enter_context(nc.semaphore("sc_sem"))
        block = ctx.enter_context(nc.Block())

        n_tiles = len(tiles)
        slot_of = [i % _N_SLOTS for i in range(n_tiles)]
        use_of = [i // _N_SLOTS for i in range(n_tiles)]
        slot_uses = [
            sum(1 for i in range(n_tiles) if slot_of[i] == s)
            for s in range(_N_SLOTS)
        ]

        @block.sync
        def _(sync):
            # scale first on the same HWDGE queue, split into column halves:
            # FIFO per SDMA engine lands half 0 before L0, so mul0 isn't
            # gated on the full 2MB broadcast row
            # scale first on the same HWDGE queue: FIFO per SDMA engine
            # guarantees it lands before L0 without costing overlap
            sync.dma_start(out=scale_sb[:, :], in_=scale[:, :]).then_inc(
                sc_sem, 16
            )
            for i, (n, c0, w) in enumerate(tiles):
                s, use = slot_of[i], use_of[i]
                if use > 0:
                    sync.wait_ge(slot_sems[s], 32 * use)  # prev store landed
                sync.dma_start(
                    out=slots[s][:, :w], in_=xv[n, :, c0 : c0 + w]
                ).then_inc(slot_sems[s], 16)

        @block.vector
        def _(vector):
            vector.wait_ge(sc_sem, 16)
            for i, (n, c0, w) in enumerate(tiles):
                s, use = slot_of[i], use_of[i]
                vector.wait_ge(slot_sems[s], 32 * use + 16)  # load landed
                t = slots[s]
                vector.tensor_mul(
                    out=t[:, :w], in0=t[:, :w], in1=scale_sb[:, c0 : c0 + w]
                ).then_inc(mul_sem, 1)

        @block.scalar
        def _(scalar):
            for i, (n, c0, w) in enumerate(tiles):
                s = slot_of[i]
                scalar.wait_ge(mul_sem, i + 1)
                scalar.dma_start(
                    out=yv[n, :, c0 : c0 + w], in_=slots[s][:, :w]
                ).then_inc(slot_sems[s], 16)
            # all output bytes landed before the program ends
            for s in range(_N_SLOTS):
                scalar.wait_ge(slot_sems[s], 32 * slot_uses[s])

    _cache["nc"] = nc
    return nc


def _run(x: np.ndarray, trace: bool = False, trace_cores=None):
    """Returns (full_output, BassKernelResults)."""
    from concourse.bass_utils import run_bass_kernel_spmd

    nc = _build_nc()
    scale_bc = np.ascontiguousarray(
        np.broadcast_to(_compute_scale()[None, :], (128, _H))
    )
    xf = np.ascontiguousarray(x, dtype=np.float32).reshape(_ROWS, _H)
    in_maps = [
        {"x": xf[c * _ROWS_PER_CORE : (c + 1) * _ROWS_PER_CORE], "scale": scale_bc}
        for c in range(_N_CORES)
    ]
    res = run_bass_kernel_spmd(
        nc,
        in_maps,
        core_ids=list(range(_N_CORES)),
        trace=trace,
        trace_cores=trace_cores,
    )
    out = np.concatenate([r["y"] for r in res.results], axis=0)
    return out.reshape(_B, _S, _H), res


def kernel(**inputs) -> np.ndarray:
    out, _ = _run(np.asarray(inputs["x"]))
    return out


# revision 34
# speedup vs baseline: 1.0259x; 1.0156x over previous
"""DropoutDPP eval-path kernel for 8 Trainium2 NeuronCores.

The reference accumulates Bernoulli keep-masks (fixed RNG key 42, independent
of x) over the hidden dim until >=40% of neurons are nonzero, then computes
x * sum_mask / i.  The mask state is a deterministic constant, so it is
computed host-side (same jax threefry bits as the reference) and folded into a
single [hidden] scale vector.  The heavy, memory-bound part — scaling the
[4, 4096, 4096] tensor along its last dim — runs on 8 cores, data-parallel
over rows.

The on-device kernel is hand-scheduled raw Bass (this toolchain's TT struct
encodes a single sync wait, which rules out Tile's multi-wait scheduling):
SP issues the scale load then x-tile loads, DVE multiplies in place, ACT
issues stores; eight 1MB SBUF slots rotate, one semaphore per slot so wait
thresholds are exact.
"""

import numpy as np

_B, _S, _H = 4, 4096, 4096
_N_CORES = 8
_ROWS = _B * _S                       # 16384
_ROWS_PER_CORE = _ROWS // _N_CORES    # 2048
_P = 0.9
_MAX_N = 100
_MAX_FRAC = 0.4

_COLS_PER_TILE = 2048                 # [128, 2048] f32 = 1MB tiles
_TILES_PER_ROWBLK = _H // _COLS_PER_TILE
_N_ROWBLKS = _ROWS_PER_CORE // 128    # 16
_N_TILES = _N_ROWBLKS * _TILES_PER_ROWBLK  # 32
_N_SLOTS = 8

_cache: dict = {}


def _compute_mask() -> tuple:
    """Replicate reference._accumulate_masks exactly (threefry is
    backend/platform deterministic): returns (sum_mask counts f32, i)."""
    if "mask" in _cache:
        return _cache["mask"]
    import jax
    import jax.numpy as jnp

    cpu = jax.devices("cpu")[0]
    with jax.default_device(cpu):
        key = jax.random.key(42)
        key, k0 = jax.random.split(key)
        sum_mask = (jax.random.uniform(k0, (_H,)) >= _P).astype(jnp.float32)
        i = 1
        while i < _MAX_N and float(
            jnp.mean((sum_mask != 0).astype(jnp.float32))
        ) < _MAX_FRAC:
            key, k = jax.random.split(key)
            sum_mask = sum_mask + (jax.random.uniform(k, (_H,)) >= _P).astype(
                jnp.float32
            )
            i += 1
    _cache["mask"] = (np.asarray(sum_mask, dtype=np.float32), i)
    return _cache["mask"]


def _compute_scale() -> np.ndarray:
    sum_mask, i = _compute_mask()
    return sum_mask / np.float32(i)


def _build_nc():
    if "nc" in _cache:
        return _cache["nc"]
    import concourse.bass as bass
    import concourse.mybir as mybir
    from contextlib import ExitStack

    nc = bass.Bass(trn_type="TRN2")
    x = nc.dram_tensor(
        "x", [_ROWS_PER_CORE, _H], mybir.dt.float32, kind="ExternalInput"
    )
    # mask counts (0..4) are bf16-exact; shipping bf16 halves the scale
    # stream, and fp32(bf16(m) * 0.2f) == fp32(m/5f) for every m here
    mask = nc.dram_tensor(
        "mask", [128, _H], mybir.dt.bfloat16, kind="ExternalInput"
    )
    y = nc.dram_tensor(
        "y", [_ROWS_PER_CORE, _H], mybir.dt.float32, kind="ExternalOutput"
    )

    # row-block-major view; tiles slice columns out of a row block
    xv = x[:, :].rearrange("(n p) h -> n p h", p=128)
    yv = y[:, :].rearrange("(n p) h -> n p h", p=128)

    # (rowblk, col0, ncols): uniform [128, 2048] 1MB tiles
    tiles = [
        (n, c * _COLS_PER_TILE, _COLS_PER_TILE)
        for n in range(_N_ROWBLKS)
        for c in range(_TILES_PER_ROWBLK)
    ]

    with ExitStack() as ctx:
        mask_sb = ctx.enter_context(
            nc.sbuf_tensor("mask_sb", [128, _H], mybir.dt.bfloat16)
        )
        scale_sb = ctx.enter_context(
            nc.sbuf_tensor("scale_sb", [128, _H], mybir.dt.float32)
        )
        slots = [
            ctx.enter_context(
                nc.sbuf_tensor(
                    f"slot{s}", [128, _COLS_PER_TILE], mybir.dt.float32
                )
            )
            for s in range(_N_SLOTS)
        ]
        # One semaphore per slot: each slot has at most one outstanding DMA
        # at a time (load +16, store +16 → +32 per slot cycle), making the
        # wait thresholds exact.  A single shared DMA sem would race: the 16
        # SDMA engines increment independently per transfer, so "sem >=
        # 16*(i+1)" does not imply transfers 0..i all completed.
        slot_sems = [
            ctx.enter_context(nc.semaphore(f"slot_sem{s}"))
            for s in range(_N_SLOTS)
        ]
        mul_sem = ctx.enter_context(nc.semaphore("mul_sem"))
        sc_sem = ctx.enter_context(nc.semaphore("sc_sem"))
        bc_sem = ctx.enter_context(nc.semaphore("bc_sem"))
        block = ctx.enter_context(nc.Block())

        n_tiles = len(tiles)
        slot_of = [i % _N_SLOTS for i in range(n_tiles)]
        use_of = [i // _N_SLOTS for i in range(n_tiles)]
        slot_uses = [
            sum(1 for i in range(n_tiles) if slot_of[i] == s)
            for s in range(_N_SLOTS)
        ]

        @block.sync
        def _(sync):
            # scale first on the same HWDGE queue, split into column halves:
            # FIFO per SDMA engine lands half 0 before L0, so mul0 isn't
            # gated on the full 2MB broadcast row
            # mask first on the same HWDGE queue: FIFO per SDMA engine
            # guarantees it lands before L0 without costing overlap
            sync.dma_start(out=mask_sb[:, :], in_=mask[:, :]).then_inc(
                sc_sem, 16
            )
            for i, (n, c0, w) in enumerate(tiles):
                s, use = slot_of[i], use_of[i]
                if use > 0:
                    sync.wait_ge(slot_sems[s], 32 * use)  # prev store landed
                sync.dma_start(
                    out=slots[s][:, :w], in_=xv[n, :, c0 : c0 + w]
                ).then_inc(slot_sems[s], 16)

        @block.vector
        def _(vector):
            # expand bf16 mask -> fp32 scale in two column halves so mul0
            # only gates on half 0; self-sem covers the DVE write->read
            # pipeline hazard before the muls consume scale_sb
            half = _H // 2
            vector.wait_ge(sc_sem, 16)
            for hno in range(2):
                cols = slice(hno * half, (hno + 1) * half)
                vector.tensor_scalar_mul(
                    scale_sb[:, cols], mask_sb[:, cols], 1.0 / _compute_mask()[1]
                ).then_inc(bc_sem, 1)
            cur_bc = 0
            for i, (n, c0, w) in enumerate(tiles):
                s, use = slot_of[i], use_of[i]
                need_bc = 1 if c0 + w <= half else 2
                if need_bc > cur_bc:
                    vector.wait_ge(bc_sem, need_bc)
                    cur_bc = need_bc
                vector.wait_ge(slot_sems[s], 32 * use + 16)  # load landed
                t = slots[s]
                vector.tensor_mul(
                    out=t[:, :w], in0=t[:, :w], in1=scale_sb[:, c0 : c0 + w]
                ).then_inc(mul_sem, 1)

        @block.scalar
        def _(scalar):
            for i, (n, c0, w) in enumerate(tiles):
                s = slot_of[i]
                scalar.wait_ge(mul_sem, i + 1)
                scalar.dma_start(
                    out=yv[n, :, c0 : c0 + w], in_=slots[s][:, :w]
                ).then_inc(slot_sems[s], 16)
            # all output bytes landed before the program ends
            for s in range(_N_SLOTS):
                scalar.wait_ge(slot_sems[s], 32 * slot_uses[s])

    _cache["nc"] = nc
    return nc


def _run(x: np.ndarray, trace: bool = False, trace_cores=None):
    """Returns (full_output, BassKernelResults)."""
    from concourse.bass_utils import run_bass_kernel_spmd

    import ml_dtypes

    nc = _build_nc()
    sum_mask, _i = _compute_mask()
    mask_bc = np.ascontiguousarray(
        np.broadcast_to(
            sum_mask.astype(ml_dtypes.bfloat16)[None, :], (128, _H)
        )
    )
    xf = np.ascontiguousarray(x, dtype=np.float32).reshape(_ROWS, _H)
    in_maps = [
        {"x": xf[c * _ROWS_PER_CORE : (c + 1) * _ROWS_PER_CORE], "mask": mask_bc}
        for c in range(_N_CORES)
    ]
    res = run_bass_kernel_spmd(
        nc,
        in_maps,
        core_ids=list(range(_N_CORES)),
        trace=trace,
        trace_cores=trace_cores,
    )
    out = np.concatenate([r["y"] for r in res.results], axis=0)
    return out.reshape(_B, _S, _H), res


def kernel(**inputs) -> np.ndarray:
    out, _ = _run(np.asarray(inputs["x"]))
    return out
